# revision 1
# baseline (speedup 1.0000x reference)
"""Trainium2 Bass kernel for nn_BranchingQNetwork (12-branch dueling Q-MLP).

Strategy: data-parallel over batch (8 cores x 1024 rows). Per core, all 12
branch MLPs run as feature-major GEMM chains (weights stationary, activations
streaming), fp32r matmuls, k-outer single-pass accumulation in 8 PSUM banks
with weights streamed through a small SBUF window. The dueling head
(v + a - mean(a)) is linear, so it is folded into a single [512, 11] weight
matrix on the host.
"""
import sys

sys.path.insert(0, "/opt/trn_rl_repo")

import numpy as np

# problem dims (hardcoded per harness contract)
B = 8192
OBS = 249
NB = 12
NA = 11
NODE = 45
GRP = 17
D0 = 62
D1 = 2048
D2 = 1024
D3 = 512

NCORES = 8
LB = B // NCORES     # local batch per core
BT = 512             # batch tile
NBT = LB // BT
M1 = D1 // 128       # 16 output tiles of layer 1
K2 = D1 // 128       # 16 contraction tiles of layer 2
M2 = D2 // 128       # 8
K3 = D2 // 128       # 8
M3 = D3 // 128       # 4
KH = D3 // 128       # 4
NCH = BT // 128      # 4 batch chunks per batch tile
NAP = 12             # head width padded even (fp32r ISA: N must be even)

_NC_CACHE = {}
LAST_RESULT = None


def _build_nc():
    if "nc" in _NC_CACHE:
        return _NC_CACHE["nc"]
    from concourse import bacc
    import concourse.mybir as mybir
    import concourse.tile as tile

    f32 = mybir.dt.float32
    f32r = mybir.dt.float32r
    Relu = mybir.ActivationFunctionType.Relu
    ADD = mybir.AluOpType.add
    MAX = mybir.AluOpType.max

    nc = bacc.Bacc("TRN2")

    xT_d = nc.declare_dram_parameter("xT", [OBS, LB], f32r, isOutput=False)
    W1_d = nc.declare_dram_parameter("W1p", [NB, D0, D1], f32r, isOutput=False)
    W2_d = nc.declare_dram_parameter("W2p", [NB, K2, 128, D2], f32r, isOutput=False)
    W3_d = nc.declare_dram_parameter("W3p", [NB, K3, 128, D3], f32r, isOutput=False)
    Wq_d = nc.declare_dram_parameter("Wqp", [NB, KH, 128, NAP], f32r, isOutput=False)
    b_d = nc.declare_dram_parameter("bp", [NB, 128, M1 + M2 + M3], f32, isOutput=False)
    bq_d = nc.declare_dram_parameter("bqp", [NB, 128, NAP], f32, isOutput=False)
    out_d = nc.declare_dram_parameter("out", [NB, LB, NA], f32, isOutput=True)

    with tile.TileContext(nc) as tc:
        with (
            tc.tile_pool(name="wp1", bufs=2) as wp1,
            tc.tile_pool(name="wp2", bufs=12) as wp2,
            tc.tile_pool(name="wp3", bufs=8) as wp3,
            tc.tile_pool(name="wpq", bufs=2) as wpq,
            tc.tile_pool(name="bbp", bufs=2) as bbp,
            tc.tile_pool(name="pxp", bufs=3) as pxp,
            tc.tile_pool(name="actp", bufs=1) as actp,
            tc.tile_pool(name="osp", bufs=3) as osp,
            tc.tile_pool(name="psp", bufs=8, space="PSUM") as psp,
        ):
            h1 = actp.tile([128, K2, BT], f32r, tag="h1")
            h2 = actp.tile([128, K3, BT], f32r, tag="h2")
            h3 = actp.tile([128, KH, BT], f32r, tag="h3")

            iters = [(br, bt) for br in range(NB) for bt in range(NBT)]
            loaded = {}
            pxs = {}

            def load_branch(br):
                eng = nc.sync if br == 0 else nc.scalar
                w1t = wp1.tile([D0, D1], f32r, tag="w1", name=f"w1_{br}")
                eng.dma_start(w1t[:], W1_d[br])
                wqt = wpq.tile([128, KH, NAP], f32r, tag="wq", name=f"wq_{br}")
                eng.dma_start(wqt[:], Wq_d[br].rearrange("k p a -> p k a"))
                btile = bbp.tile([128, M1 + M2 + M3], f32, tag="b", name=f"b_{br}")
                eng.dma_start(btile[:], b_d[br])
                bqt = bbp.tile([128, NAP], f32, tag="bq", name=f"bq_{br}")
                eng.dma_start(bqt[:], bq_d[br])
                loaded[br] = (w1t, wqt, btile, bqt)

            def load_px(idx):
                br, bt = iters[idx]
                eng = nc.sync if idx == 0 else nc.scalar
                bsl = slice(bt * BT, (bt + 1) * BT)
                px = pxp.tile([D0, BT], f32r, tag="px", name=f"px_{idx}")
                eng.dma_start(px[0:NODE, :], xT_d[0:NODE, bsl])
                g0 = NODE + GRP * br
                eng.dma_start(px[NODE:D0, :], xT_d[g0:g0 + GRP, bsl])
                pxs[idx] = px

            def drain(dst, ps, bias, j):
                if j % 2 == 0:
                    nc.scalar.activation(dst, ps, Relu, bias=bias, scale=1.0)
                else:
                    nc.vector.tensor_scalar(dst, ps, bias, 0.0, ADD, MAX)

            def drain_split(h, m, ps, bias, j):
                half = BT // 2
                a = h[:, m, 0:half]
                b = h[:, m, half:BT]
                pa = ps[:, 0:half]
                pb = ps[:, half:BT]
                if j % 2 == 0:
                    nc.scalar.activation(a, pa, Relu, bias=bias, scale=1.0)
                    nc.vector.tensor_scalar(b, pb, bias, 0.0, ADD, MAX)
                else:
                    nc.vector.tensor_scalar(a, pa, bias, 0.0, ADD, MAX)
                    nc.scalar.activation(b, pb, Relu, bias=bias, scale=1.0)

            def emit_L1_mm(idx, m):
                br, _ = iters[idx]
                w1t, _, btile, _ = loaded[br]
                ps = psp.tile([128, BT], f32, tag="ps", name=f"l1ps_{idx}_{m}")
                nc.tensor.matmul(
                    ps[:], w1t[:, m * 128:(m + 1) * 128], pxs[idx][:],
                    start=True, stop=True,
                )
                drain(h1[:, m, :], ps[:], btile[:, m:m + 1], m)

            # prologue: first iteration's L1 runs standalone
            load_branch(0)
            load_px(0)
            for m in range(M1):
                emit_L1_mm(0, m)

            for idx, (br, bt) in enumerate(iters):
                w1t, wqt, btile, bqt = loaded[br]
                nxt = idx + 1
                if nxt < len(iters):
                    nbr = iters[nxt][0]
                    if nbr not in loaded:
                        load_branch(nbr)
                    load_px(nxt)

                # ---- L2: [2048 -> 1024], k-outer, 8 psum banks ----
                ps2 = [psp.tile([128, BT], f32, tag="ps", name=f"ps2_{idx}_{_m}")
                       for _m in range(M2)]
                for k in range(K2):
                    w2t = wp2.tile([128, D2], f32r, tag="w2", name=f"w2_{idx}_{k}")
                    nc.sync.dma_start(w2t[:], W2_d[br, k])
                    for m in range(M2):
                        nc.tensor.matmul(
                            ps2[m][:], w2t[:, m * 128:(m + 1) * 128],
                            h1[:, k, :],
                            start=(k == 0), stop=(k == K2 - 1),
                        )
                for m in range(M2):
                    drain(h2[:, m, :], ps2[m][:], btile[:, M1 + m:M1 + m + 1], m)

                # ---- L3 [1024 -> 512] interleaved with next iteration's L1 ----
                ps3 = [psp.tile([128, BT], f32, tag="ps", name=f"ps3_{idx}_{_m}")
                       for _m in range(M3)]
                for k in range(K3):
                    w3t = wp3.tile([128, D3], f32r, tag="w3", name=f"w3_{idx}_{k}")
                    nc.sync.dma_start(w3t[:], W3_d[br, k])
                    for m in range(M3):
                        nc.tensor.matmul(
                            ps3[m][:], w3t[:, m * 128:(m + 1) * 128],
                            h2[:, k, :],
                            start=(k == 0), stop=(k == K3 - 1),
                        )
                    if nxt < len(iters):
                        emit_L1_mm(nxt, 2 * k)
                        emit_L1_mm(nxt, 2 * k + 1)
                for m in range(M3):
                    drain(h3[:, m, :], ps3[m][:],
                          btile[:, M1 + M2 + m:M1 + M2 + m + 1], m)

                # ---- head: q = h3 @ Wq + bq, batch-major out ----
                bsl = slice(bt * BT, (bt + 1) * BT)
                ost = osp.tile([128, NCH * NAP], f32, tag="os", name=f"ost_{idx}")
                for c in range(NCH):
                    psh = psp.tile([128, NAP], f32, tag="ps", name=f"psh_{idx}_{c}")
                    for k in range(KH):
                        nc.tensor.matmul(
                            psh[:], h3[:, k, c * 128:(c + 1) * 128],
                            wqt[:, k, :],
                            start=(k == 0), stop=(k == KH - 1),
                        )
                    nc.vector.tensor_add(ost[:, c * NAP:(c + 1) * NAP],
                                         psh[:], bqt[:])
                nc.gpsimd.dma_start(
                    out_d[br, bsl, :].rearrange("(c p) a -> p c a", p=128),
                    ost[:].rearrange("p (c a) -> p c a", c=NCH)[:, :, 0:NA],
                )

    nc.compile()
    _NC_CACHE["nc"] = nc
    return nc


def _pack_weights(W1, b1, W2, b2, W3, b3, Wv, bv, Wa, ba):
    f = np.float32
    W1p = np.ascontiguousarray(W1, dtype=f)                      # [12, 62, 2048]
    W2p = np.ascontiguousarray(W2.reshape(NB, K2, 128, D2), f)   # [12,16,128,1024]
    W3p = np.ascontiguousarray(W3.reshape(NB, K3, 128, D3), f)   # [12,8,128,512]
    # fold dueling head: q = h @ (Wv + Wa - mean(Wa)) + (bv + ba - mean(ba))
    Wq = Wv + Wa - Wa.mean(axis=2, keepdims=True)                # [12, 512, 11]
    bq = bv + ba - ba.mean(axis=1, keepdims=True)                # [12, 11]
    Wq = np.concatenate([Wq, np.zeros((NB, D3, NAP - NA), Wq.dtype)], axis=2)
    bq = np.concatenate([bq, np.zeros((NB, NAP - NA), bq.dtype)], axis=1)
    Wqp = np.ascontiguousarray(Wq.reshape(NB, KH, 128, NAP), f)
    bp = np.concatenate(
        [
            b1.reshape(NB, M1, 128).transpose(0, 2, 1),
            b2.reshape(NB, M2, 128).transpose(0, 2, 1),
            b3.reshape(NB, M3, 128).transpose(0, 2, 1),
        ],
        axis=2,
    ).astype(f)                                                  # [12, 128, 28]
    bqp = np.ascontiguousarray(
        np.broadcast_to(bq[:, None, :], (NB, 128, NAP)), f
    )
    return W1p, W2p, W3p, Wqp, bp, bqp


def kernel(x, W1, b1, W2, b2, W3, b3, Wv, bv, Wa, ba):
    global LAST_RESULT
    from concourse.bass_utils import run_bass_kernel_spmd

    x = np.asarray(x, np.float32)
    args = [np.asarray(a, np.float32) for a in (W1, b1, W2, b2, W3, b3, Wv, bv, Wa, ba)]
    W1p, W2p, W3p, Wqp, bp, bqp = _pack_weights(*args)

    nc = _build_nc()
    in_maps = []
    for c in range(NCORES):
        xT = np.ascontiguousarray(x[c * LB:(c + 1) * LB].T)      # [249, 1024]
        in_maps.append({
            "xT": xT,
            "W1p": W1p, "W2p": W2p, "W3p": W3p, "Wqp": Wqp,
            "bp": bp, "bqp": bqp,
        })

    res = run_bass_kernel_spmd(nc, in_maps, list(range(NCORES)))
    LAST_RESULT = res

    out = np.empty((NB, B, NA), np.float32)
    for c in range(NCORES):
        out[:, c * LB:(c + 1) * LB, :] = res.results[c]["out"]
    return out



# revision 3
# speedup vs baseline: 1.0966x; 1.0966x over previous
"""Trainium2 Bass kernel for nn_BranchingQNetwork (12-branch dueling Q-MLP).

Strategy: hybrid sharding — 4 branch-groups x 2 batch-halves over 8 cores.
Each core computes 3 branches for 4096 rows. All tensors bf16 (weights and
activations; fp32 PSUM accumulation), so the full 3-branch weight set
(~16 MB) is SBUF-resident, prefetched one branch ahead; matmuls never wait
on HBM. The dueling head (v + a - mean(a)) is linear and folded into a
single [512, 12] weight on the host; head output is produced action-major
[12, batch] on-chip and transposed on the host.

PSUM discipline: 4 "accumulator" banks (L2 m-groups of 4, then L3) +
4 "streaming" banks (L1 single-shot matmuls, head). L1 matmuls of the next
iteration are emitted as fillers around the L2/L3 accumulation phases to
hide PSUM-drain latency.
"""
import sys

sys.path.insert(0, "/opt/trn_rl_repo")

import numpy as np
import ml_dtypes

BF16 = ml_dtypes.bfloat16

# problem dims (hardcoded per harness contract)
B = 8192
OBS = 249
NB = 12
NA = 11
NODE = 45
GRP = 17
D0 = 62
D1 = 2048
D2 = 1024
D3 = 512

NCORES = 8
NGRP = 4             # branch groups
NBL = NB // NGRP     # 3 branches per core
LBH = B // 2         # 4096 rows per core (batch halves)
BT = 512             # batch tile
NBT = LBH // BT      # 8 batch tiles
M1 = D1 // 128       # 16 output tiles of layer 1
K2 = D1 // 128       # 16 contraction tiles of layer 2
M2 = D2 // 128       # 8
K3 = D2 // 128       # 8
M3 = D3 // 128       # 4
KH = D3 // 128       # 4
NAP = 12             # head width padded even

_NC_CACHE = {}
LAST_RESULT = None


def _build_nc():
    if "nc" in _NC_CACHE:
        return _NC_CACHE["nc"]
    from concourse import bacc
    import concourse.mybir as mybir
    import concourse.tile as tile

    f32 = mybir.dt.float32
    bf16 = mybir.dt.bfloat16
    Relu = mybir.ActivationFunctionType.Relu
    ADD = mybir.AluOpType.add
    MAX = mybir.AluOpType.max

    nc = bacc.Bacc("TRN2")

    px_d = nc.declare_dram_parameter("pxp", [NBL, D0, LBH], bf16, isOutput=False)
    W1_d = nc.declare_dram_parameter("W1p", [NBL, D0, D1], bf16, isOutput=False)
    W2_d = nc.declare_dram_parameter("W2p", [NBL, 128, K2, D2], bf16, isOutput=False)
    W3_d = nc.declare_dram_parameter("W3p", [NBL, 128, K3, D3], bf16, isOutput=False)
    Wq_d = nc.declare_dram_parameter("Wqp", [NBL, 128, KH, NAP], bf16, isOutput=False)
    b_d = nc.declare_dram_parameter("bp", [NBL, 128, M1 + M2 + M3], f32, isOutput=False)
    bq_d = nc.declare_dram_parameter("bqp", [NBL, NAP, 1], f32, isOutput=False)
    out_d = nc.declare_dram_parameter("out", [NBL, NAP, LBH], f32, isOutput=True)

    with tile.TileContext(nc) as tc:
        with (
            tc.tile_pool(name="wp1", bufs=2) as wp1,
            tc.tile_pool(name="wp2", bufs=2) as wp2,
            tc.tile_pool(name="wp3", bufs=2) as wp3,
            tc.tile_pool(name="wpq", bufs=2) as wpq,
            tc.tile_pool(name="bbp", bufs=2) as bbp,
            tc.tile_pool(name="pxp_s", bufs=3) as pxp,
            tc.tile_pool(name="h1p", bufs=2) as h1p,
            tc.tile_pool(name="actp", bufs=1) as actp,
            tc.tile_pool(name="osp", bufs=3) as osp,
            tc.tile_pool(name="psA", bufs=4, space="PSUM") as psA,
            tc.tile_pool(name="psS", bufs=4, space="PSUM") as psS,
        ):
            h2 = actp.tile([128, K3, BT], bf16, tag="h2")
            h3 = actp.tile([128, KH, BT], bf16, tag="h3")

            iters = [(br, bt) for br in range(NBL) for bt in range(NBT)]
            loaded = {}
            pxs = {}
            h1s = {}

            def load_branch(br):
                w1t = wp1.tile([D0, D1], bf16, tag="w1", name=f"w1_{br}")
                nc.sync.dma_start(w1t[:], W1_d[br])
                w2t = wp2.tile([128, K2, D2], bf16, tag="w2", name=f"w2_{br}")
                for c in range(4):
                    nc.sync.dma_start(
                        w2t[:, 4 * c:4 * c + 4, :], W2_d[br][:, 4 * c:4 * c + 4, :]
                    )
                w3t = wp3.tile([128, K3, D3], bf16, tag="w3", name=f"w3_{br}")
                nc.sync.dma_start(w3t[:], W3_d[br])
                wqt = wpq.tile([128, KH, NAP], bf16, tag="wq", name=f"wq_{br}")
                nc.sync.dma_start(wqt[:], Wq_d[br])
                btile = bbp.tile([128, M1 + M2 + M3], f32, tag="b", name=f"b_{br}")
                nc.sync.dma_start(btile[:], b_d[br])
                bqt = bbp.tile([NAP, 1], f32, tag="bq", name=f"bq_{br}")
                nc.sync.dma_start(bqt[:], bq_d[br])
                loaded[br] = (w1t, w2t, w3t, wqt, btile, bqt)

            def load_px(idx):
                br, bt = iters[idx]
                bsl = slice(bt * BT, (bt + 1) * BT)
                px = pxp.tile([D0, BT], bf16, tag="px", name=f"px_{idx}")
                nc.scalar.dma_start(px[:], px_d[br][:, bsl])
                pxs[idx] = px
                h1s[idx] = h1p.tile([128, K2, BT], bf16, tag="h1", name=f"h1_{idx}")

            def drain_split(dst, ps, bias, j):
                half = BT // 2
                a, b = dst[:, 0:half], dst[:, half:BT]
                pa, pb = ps[:, 0:half], ps[:, half:BT]
                if j % 2 == 0:
                    nc.scalar.activation(a, pa, Relu, bias=bias, scale=1.0)
                    nc.vector.tensor_scalar(b, pb, bias, 0.0, ADD, MAX)
                else:
                    nc.vector.tensor_scalar(a, pa, bias, 0.0, ADD, MAX)
                    nc.scalar.activation(b, pb, Relu, bias=bias, scale=1.0)

            def emit_L1_mm(idx, m):
                br, _ = iters[idx]
                w1t, _, _, _, btile, _ = loaded[br]
                ps = psS.tile([128, BT], f32, tag="ps", name=f"l1ps_{idx}_{m}")
                nc.tensor.matmul(
                    ps[:], w1t[:, m * 128:(m + 1) * 128], pxs[idx][:],
                    start=True, stop=True,
                )
                drain_split(h1s[idx][:, m, :], ps[:], btile[:, m:m + 1], m)

            def emit_head(idx):
                br, bt = iters[idx]
                _, _, _, wqt, _, bqt = loaded[br]
                psh = psS.tile([NAP, BT], f32, tag="ps", name=f"psh_{idx}")
                for k in range(KH):
                    nc.tensor.matmul(
                        psh[:], wqt[:, k, :], h3[:, k, :],
                        start=(k == 0), stop=(k == KH - 1),
                    )
                ot = osp.tile([NAP, BT], f32, tag="os", name=f"ot_{idx}")
                nc.vector.tensor_scalar_add(ot[:], psh[:], bqt[:])
                bsl = slice(bt * BT, (bt + 1) * BT)
                nc.gpsimd.dma_start(out_d[br][:, bsl], ot[:])

            # prologue: first branch + first px, L1 of iteration 0
            load_branch(0)
            load_px(0)
            for m in range(M1):
                emit_L1_mm(0, m)

            for idx, (br, bt) in enumerate(iters):
                w1t, w2t, w3t, wqt, btile, bqt = loaded[br]
                h1 = h1s[idx]
                nxt = idx + 1
                have_nxt = nxt < len(iters)
                if have_nxt:
                    load_px(nxt)
                if bt == 0 and br + 1 < NBL:
                    load_branch(br + 1)

                # ---- L2 [2048 -> 1024]: two m-groups of 4, k-outer ----
                for mg in range(2):
                    ps2 = [psA.tile([128, BT], f32, tag="ps",
                                    name=f"ps2_{idx}_{mg}_{j}") for j in range(4)]
                    for k in range(K2):
                        for j in range(4):
                            m = mg * 4 + j
                            nc.tensor.matmul(
                                ps2[j][:], w2t[:, k, m * 128:(m + 1) * 128],
                                h1[:, k, :],
                                start=(k == 0), stop=(k == K2 - 1),
                            )
                        if mg == 0 and k == 0 and idx > 0:
                            emit_head(idx - 1)  # prev head hides in mg0
                    for j in range(4):
                        m = mg * 4 + j
                        drain_split(h2[:, m, :], ps2[j][:],
                                    btile[:, M1 + m:M1 + m + 1], j)
                    # L1 fillers hide the m-group drain latency
                    if have_nxt:
                        emit_L1_mm(nxt, 2 * mg)
                        emit_L1_mm(nxt, 2 * mg + 1)

                # ---- L3 [1024 -> 512] k-outer + next L1 interleave ----
                ps3 = [psA.tile([128, BT], f32, tag="ps",
                                name=f"ps3_{idx}_{m}") for m in range(M3)]
                for k in range(K3):
                    for m in range(M3):
                        nc.tensor.matmul(
                            ps3[m][:], w3t[:, k, m * 128:(m + 1) * 128],
                            h2[:, k, :],
                            start=(k == 0), stop=(k == K3 - 1),
                        )
                    if have_nxt and k < 5:
                        emit_L1_mm(nxt, 4 + 2 * k)
                        emit_L1_mm(nxt, 5 + 2 * k)
                if have_nxt:
                    emit_L1_mm(nxt, 14)
                    emit_L1_mm(nxt, 15)
                for m in range(M3):
                    drain_split(h3[:, m, :], ps3[m][:],
                                btile[:, M1 + M2 + m:M1 + M2 + m + 1], m)

            emit_head(len(iters) - 1)

    nc.compile()
    _NC_CACHE["nc"] = nc
    return nc


def _pack_weights(W1, b1, W2, b2, W3, b3, Wv, bv, Wa, ba):
    f = np.float32
    W1p = np.ascontiguousarray(W1).astype(BF16)                   # [12, 62, 2048]
    W2p = np.ascontiguousarray(
        W2.reshape(NB, K2, 128, D2).transpose(0, 2, 1, 3)).astype(BF16)
    W3p = np.ascontiguousarray(
        W3.reshape(NB, K3, 128, D3).transpose(0, 2, 1, 3)).astype(BF16)
    # fold dueling head: q = h @ (Wv + Wa - mean(Wa)) + (bv + ba - mean(ba))
    Wq = Wv + Wa - Wa.mean(axis=2, keepdims=True)                 # [12, 512, 11]
    bq = bv + ba - ba.mean(axis=1, keepdims=True)                 # [12, 11]
    Wq = np.concatenate([Wq, np.zeros((NB, D3, NAP - NA), Wq.dtype)], axis=2)
    bq = np.concatenate([bq, np.zeros((NB, NAP - NA), bq.dtype)], axis=1)
    Wqp = np.ascontiguousarray(
        Wq.reshape(NB, KH, 128, NAP).transpose(0, 2, 1, 3)).astype(BF16)
    bp = np.concatenate(
        [
            b1.reshape(NB, M1, 128).transpose(0, 2, 1),
            b2.reshape(NB, M2, 128).transpose(0, 2, 1),
            b3.reshape(NB, M3, 128).transpose(0, 2, 1),
        ],
        axis=2,
    ).astype(f)                                                   # [12, 128, 28]
    bqp = np.ascontiguousarray(bq[:, :, None], f)                 # [12, 12, 1]
    return W1p, W2p, W3p, Wqp, bp, bqp


def kernel(x, W1, b1, W2, b2, W3, b3, Wv, bv, Wa, ba):
    global LAST_RESULT
    from concourse.bass_utils import run_bass_kernel_spmd

    x = np.asarray(x, np.float32)
    args = [np.asarray(a, np.float32) for a in (W1, b1, W2, b2, W3, b3, Wv, bv, Wa, ba)]
    W1p, W2p, W3p, Wqp, bp, bqp = _pack_weights(*args)

    node = x[:, :NODE]                                  # [B, 45]
    groups = x[:, NODE:].reshape(B, NB, GRP)            # [B, 12, 17]

    nc = _build_nc()
    in_maps = []
    for c in range(NCORES):
        g, h = c // 2, c % 2
        rows = slice(h * LBH, (h + 1) * LBH)
        brs = slice(g * NBL, (g + 1) * NBL)
        pxc = np.empty((NBL, D0, LBH), BF16)
        nT = np.ascontiguousarray(node[rows].T).astype(BF16)
        for j in range(NBL):
            pxc[j, :NODE] = nT
            pxc[j, NODE:] = groups[rows, g * NBL + j].T
        in_maps.append({
            "pxp": pxc,
            "W1p": W1p[brs], "W2p": W2p[brs], "W3p": W3p[brs], "Wqp": Wqp[brs],
            "bp": bp[brs], "bqp": bqp[brs],
        })

    res = run_bass_kernel_spmd(nc, in_maps, list(range(NCORES)))
    LAST_RESULT = res

    out = np.empty((NB, B, NA), np.float32)
    for c in range(NCORES):
        g, h = c // 2, c % 2
        rows = slice(h * LBH, (h + 1) * LBH)
        oc = res.results[c]["out"]                      # [3, 12, 4096]
        for j in range(NBL):
            out[g * NBL + j, rows, :] = oc[j, :NA, :].T
    return out


# revision 8
# speedup vs baseline: 1.1346x; 1.0346x over previous
"""Trainium2 Bass kernel for nn_BranchingQNetwork (12-branch dueling Q-MLP).

Strategy: hybrid sharding — 4 branch-groups x 2 batch-halves over 8 cores.
Each core computes 3 branches for 4096 rows. All tensors bf16 (weights and
activations; fp32 PSUM accumulation), so the full 3-branch weight set
(~16 MB) is SBUF-resident, prefetched one branch ahead; matmuls never wait
on HBM. The dueling head (v + a - mean(a)) is linear and folded into a
single [512, 12] weight on the host; head output is produced action-major
[12, batch] on-chip and transposed on the host.

PSUM discipline: 4 "accumulator" banks (L2 m-groups of 4, then L3) +
4 "streaming" banks (L1 single-shot matmuls, head). L1 matmuls of the next
iteration are emitted as fillers around the L2/L3 accumulation phases to
hide PSUM-drain latency.
"""
import sys

sys.path.insert(0, "/opt/trn_rl_repo")

import numpy as np
import ml_dtypes

BF16 = ml_dtypes.bfloat16

# problem dims (hardcoded per harness contract)
B = 8192
OBS = 249
NB = 12
NA = 11
NODE = 45
GRP = 17
D0 = 62
D0P = 128            # L1 contraction padded to full PE height (avoids row_grp
                     # switches between 62-row and 128-row stationaries, which
                     # expose LDWEIGHTS on every L1<->L2/L3 transition)
D1 = 2048
D2 = 1024
D3 = 512

NCORES = 8
NGRP = 4             # branch groups
NBL = NB // NGRP     # 3 branches per core
LBH = B // 2         # 4096 rows per core (batch halves)
BT = 512             # batch tile
NBT = LBH // BT      # 8 batch tiles
M1 = D1 // 128       # 16 output tiles of layer 1
K2 = D1 // 128       # 16 contraction tiles of layer 2
M2 = D2 // 128       # 8
K3 = D2 // 128       # 8
M3 = D3 // 128       # 4
KH = D3 // 128       # 4
NAP = 12             # head width padded even

_NC_CACHE = {}
LAST_RESULT = None


def _build_nc():
    if "nc" in _NC_CACHE:
        return _NC_CACHE["nc"]
    from concourse import bacc
    import concourse.mybir as mybir
    import concourse.tile as tile

    f32 = mybir.dt.float32
    bf16 = mybir.dt.bfloat16
    Relu = mybir.ActivationFunctionType.Relu
    ADD = mybir.AluOpType.add
    MAX = mybir.AluOpType.max

    nc = bacc.Bacc("TRN2")

    px_d = nc.declare_dram_parameter("pxp", [NBL, D0P, LBH], bf16, isOutput=False)
    W1_d = nc.declare_dram_parameter("W1p", [NBL, D0P, D1], bf16, isOutput=False)
    W2_d = nc.declare_dram_parameter("W2p", [NBL, 128, K2, D2], bf16, isOutput=False)
    W3_d = nc.declare_dram_parameter("W3p", [NBL, 128, K3, D3], bf16, isOutput=False)
    Wq_d = nc.declare_dram_parameter("Wqp", [NBL, 128, KH, NAP], bf16, isOutput=False)
    b_d = nc.declare_dram_parameter("bp", [NBL, 128, M1 + M2 + M3], f32, isOutput=False)
    bq_d = nc.declare_dram_parameter("bqp", [NBL, NAP, 1], f32, isOutput=False)
    out_d = nc.declare_dram_parameter("out", [NBL, NAP, LBH], f32, isOutput=True)

    with tile.TileContext(nc) as tc:
        with (
            tc.tile_pool(name="wp1", bufs=2) as wp1,
            tc.tile_pool(name="wp2", bufs=2) as wp2,
            tc.tile_pool(name="wp3", bufs=2) as wp3,
            tc.tile_pool(name="wpq", bufs=2) as wpq,
            tc.tile_pool(name="bbp", bufs=2) as bbp,
            tc.tile_pool(name="pxp_s", bufs=3) as pxp,
            tc.tile_pool(name="h1p", bufs=2) as h1p,
            tc.tile_pool(name="actp", bufs=1) as actp,
            tc.tile_pool(name="osp", bufs=3) as osp,
            tc.tile_pool(name="psA", bufs=4, space="PSUM") as psA,
            tc.tile_pool(name="psS", bufs=4, space="PSUM") as psS,
        ):
            h2 = actp.tile([128, K3, BT], bf16, tag="h2")
            h3 = actp.tile([128, KH, BT], bf16, tag="h3")

            iters = [(br, bt) for br in range(NBL) for bt in range(NBT)]
            loaded = {}
            pxs = {}
            h1s = {}

            def load_branch(br):
                # branch 0 loads gate the pipeline start: spread them over
                # idle queues so W1/px land first and W2 chunks in parallel
                first = br == 0
                w_eng = [nc.gpsimd, nc.scalar, nc.sync, nc.gpsimd] if first \
                    else [nc.sync] * 4
                w1t = wp1.tile([D0P, D1], bf16, tag="w1", name=f"w1_{br}")
                nc.sync.dma_start(w1t[:], W1_d[br])
                w2t = wp2.tile([128, K2, D2], bf16, tag="w2", name=f"w2_{br}")
                for c in range(4):
                    w_eng[c].dma_start(
                        w2t[:, 4 * c:4 * c + 4, :], W2_d[br][:, 4 * c:4 * c + 4, :]
                    )
                w3t = wp3.tile([128, K3, D3], bf16, tag="w3", name=f"w3_{br}")
                (nc.scalar if first else nc.sync).dma_start(w3t[:], W3_d[br])
                wqt = wpq.tile([128, KH, NAP], bf16, tag="wq", name=f"wq_{br}")
                nc.sync.dma_start(wqt[:], Wq_d[br])
                btile = bbp.tile([128, M1 + M2 + M3], f32, tag="b", name=f"b_{br}")
                nc.sync.dma_start(btile[:], b_d[br])
                bqt = bbp.tile([NAP, 1], f32, tag="bq", name=f"bq_{br}")
                nc.sync.dma_start(bqt[:], bq_d[br])
                loaded[br] = (w1t, w2t, w3t, wqt, btile, bqt)

            def load_px(idx):
                br, bt = iters[idx]
                bsl = slice(bt * BT, (bt + 1) * BT)
                px = pxp.tile([D0P, BT], bf16, tag="px", name=f"px_{idx}")
                nc.scalar.dma_start(px[:], px_d[br][:, bsl])
                pxs[idx] = px
                h1s[idx] = h1p.tile([128, K2, BT], bf16, tag="h1", name=f"h1_{idx}")

            def drain_split(dst, ps, bias, j):
                half = BT // 2
                a, b = dst[:, 0:half], dst[:, half:BT]
                pa, pb = ps[:, 0:half], ps[:, half:BT]
                if j % 2 == 0:
                    nc.scalar.activation(a, pa, Relu, bias=bias, scale=1.0)
                    nc.vector.tensor_scalar(b, pb, bias, 0.0, ADD, MAX)
                else:
                    nc.vector.tensor_scalar(a, pa, bias, 0.0, ADD, MAX)
                    nc.scalar.activation(b, pb, Relu, bias=bias, scale=1.0)

            def emit_L1_mm(idx, m):
                br, _ = iters[idx]
                w1t, _, _, _, btile, _ = loaded[br]
                ps = psS.tile([128, BT], f32, tag="ps", name=f"l1ps_{idx}_{m}")
                nc.tensor.matmul(
                    ps[:], w1t[:, m * 128:(m + 1) * 128], pxs[idx][:],
                    start=True, stop=True,
                )
                drain_split(h1s[idx][:, m, :], ps[:], btile[:, m:m + 1], m)

            def emit_head(idx):
                br, bt = iters[idx]
                _, _, _, wqt, _, bqt = loaded[br]
                psh = psS.tile([NAP, BT], f32, tag="ps", name=f"psh_{idx}")
                for k in range(KH):
                    nc.tensor.matmul(
                        psh[:], wqt[:, k, :], h3[:, k, :],
                        start=(k == 0), stop=(k == KH - 1),
                    )
                ot = osp.tile([NAP, BT], f32, tag="os", name=f"ot_{idx}")
                nc.vector.tensor_scalar_add(ot[:], psh[:], bqt[:])
                bsl = slice(bt * BT, (bt + 1) * BT)
                nc.gpsimd.dma_start(out_d[br][:, bsl], ot[:])

            # prologue: first branch + first px, L1 of iteration 0
            load_branch(0)
            load_px(0)
            for m in range(M1):
                emit_L1_mm(0, m)

            for idx, (br, bt) in enumerate(iters):
                w1t, w2t, w3t, wqt, btile, bqt = loaded[br]
                h1 = h1s[idx]
                nxt = idx + 1
                have_nxt = nxt < len(iters)
                if have_nxt:
                    load_px(nxt)
                if bt == 0 and br + 1 < NBL:
                    load_branch(br + 1)

                # ---- L2 [2048 -> 1024]: two m-groups of 4, k-outer ----
                for mg in range(2):
                    ps2 = [psA.tile([128, BT], f32, tag="ps",
                                    name=f"ps2_{idx}_{mg}_{j}") for j in range(4)]
                    for k in range(K2):
                        for j in range(4):
                            m = mg * 4 + j
                            nc.tensor.matmul(
                                ps2[j][:], w2t[:, k, m * 128:(m + 1) * 128],
                                h1[:, k, :],
                                start=(k == 0), stop=(k == K2 - 1),
                            )
                        if mg == 0 and k == 0 and idx > 0:
                            emit_head(idx - 1)  # prev head hides in mg0
                    for j in range(4):
                        m = mg * 4 + j
                        drain_split(h2[:, m, :], ps2[j][:],
                                    btile[:, M1 + m:M1 + m + 1], j)
                    # L1 fillers hide the m-group drain latency
                    if have_nxt:
                        emit_L1_mm(nxt, 2 * mg)
                        emit_L1_mm(nxt, 2 * mg + 1)

                # ---- L3 [1024 -> 512] k-outer + next L1 interleave ----
                ps3 = [psA.tile([128, BT], f32, tag="ps",
                                name=f"ps3_{idx}_{m}") for m in range(M3)]
                for k in range(K3):
                    for m in range(M3):
                        nc.tensor.matmul(
                            ps3[m][:], w3t[:, k, m * 128:(m + 1) * 128],
                            h2[:, k, :],
                            start=(k == 0), stop=(k == K3 - 1),
                        )
                    if have_nxt and k < 5:
                        emit_L1_mm(nxt, 4 + 2 * k)
                        emit_L1_mm(nxt, 5 + 2 * k)
                if have_nxt:
                    emit_L1_mm(nxt, 14)
                    emit_L1_mm(nxt, 15)
                for m in range(M3):
                    drain_split(h3[:, m, :], ps3[m][:],
                                btile[:, M1 + M2 + m:M1 + M2 + m + 1], m)

            emit_head(len(iters) - 1)

    nc.compile()
    _NC_CACHE["nc"] = nc
    return nc


def _pack_weights(W1, b1, W2, b2, W3, b3, Wv, bv, Wa, ba):
    f = np.float32
    W1p = np.zeros((NB, D0P, D1), BF16)                           # [12, 128, 2048]
    W1p[:, :D0, :] = np.ascontiguousarray(W1).astype(BF16)
    W2p = np.ascontiguousarray(
        W2.reshape(NB, K2, 128, D2).transpose(0, 2, 1, 3)).astype(BF16)
    W3p = np.ascontiguousarray(
        W3.reshape(NB, K3, 128, D3).transpose(0, 2, 1, 3)).astype(BF16)
    # fold dueling head: q = h @ (Wv + Wa - mean(Wa)) + (bv + ba - mean(ba))
    Wq = Wv + Wa - Wa.mean(axis=2, keepdims=True)                 # [12, 512, 11]
    bq = bv + ba - ba.mean(axis=1, keepdims=True)                 # [12, 11]
    Wq = np.concatenate([Wq, np.zeros((NB, D3, NAP - NA), Wq.dtype)], axis=2)
    bq = np.concatenate([bq, np.zeros((NB, NAP - NA), bq.dtype)], axis=1)
    Wqp = np.ascontiguousarray(
        Wq.reshape(NB, KH, 128, NAP).transpose(0, 2, 1, 3)).astype(BF16)
    bp = np.concatenate(
        [
            b1.reshape(NB, M1, 128).transpose(0, 2, 1),
            b2.reshape(NB, M2, 128).transpose(0, 2, 1),
            b3.reshape(NB, M3, 128).transpose(0, 2, 1),
        ],
        axis=2,
    ).astype(f)                                                   # [12, 128, 28]
    bqp = np.ascontiguousarray(bq[:, :, None], f)                 # [12, 12, 1]
    return W1p, W2p, W3p, Wqp, bp, bqp


def kernel(x, W1, b1, W2, b2, W3, b3, Wv, bv, Wa, ba):
    global LAST_RESULT
    from concourse.bass_utils import run_bass_kernel_spmd

    x = np.asarray(x, np.float32)
    args = [np.asarray(a, np.float32) for a in (W1, b1, W2, b2, W3, b3, Wv, bv, Wa, ba)]
    W1p, W2p, W3p, Wqp, bp, bqp = _pack_weights(*args)

    node = x[:, :NODE]                                  # [B, 45]
    groups = x[:, NODE:].reshape(B, NB, GRP)            # [B, 12, 17]

    nc = _build_nc()
    in_maps = []
    for c in range(NCORES):
        g, h = c // 2, c % 2
        rows = slice(h * LBH, (h + 1) * LBH)
        brs = slice(g * NBL, (g + 1) * NBL)
        pxc = np.zeros((NBL, D0P, LBH), BF16)
        nT = np.ascontiguousarray(node[rows].T).astype(BF16)
        for j in range(NBL):
            pxc[j, :NODE] = nT
            pxc[j, NODE:D0] = groups[rows, g * NBL + j].T
        in_maps.append({
            "pxp": pxc,
            "W1p": W1p[brs], "W2p": W2p[brs], "W3p": W3p[brs], "Wqp": Wqp[brs],
            "bp": bp[brs], "bqp": bqp[brs],
        })

    res = run_bass_kernel_spmd(nc, in_maps, list(range(NCORES)))
    LAST_RESULT = res

    out = np.empty((NB, B, NA), np.float32)
    for c in range(NCORES):
        g, h = c // 2, c % 2
        rows = slice(h * LBH, (h + 1) * LBH)
        oc = res.results[c]["out"]                      # [3, 12, 4096]
        for j in range(NBL):
            out[g * NBL + j, rows, :] = oc[j, :NA, :].T
    return out


# revision 13
# speedup vs baseline: 1.1366x; 1.0018x over previous
"""Trainium2 Bass kernel for nn_BranchingQNetwork (12-branch dueling Q-MLP).

Strategy: hybrid sharding — 4 branch-groups x 2 batch-halves over 8 cores.
Each core computes 3 branches for 4096 rows. All tensors bf16 (weights and
activations; fp32 PSUM accumulation), so the full 3-branch weight set
(~16 MB) is SBUF-resident, prefetched one branch ahead; matmuls never wait
on HBM. The dueling head (v + a - mean(a)) is linear and folded into a
single [512, 12] weight on the host; head output is produced action-major
[12, batch] on-chip and transposed on the host.

PSUM discipline: 4 "accumulator" banks (L2 m-groups of 4, then L3) +
4 "streaming" banks (L1 single-shot matmuls, head). L1 matmuls of the next
iteration are emitted as fillers around the L2/L3 accumulation phases to
hide PSUM-drain latency.
"""
import sys

sys.path.insert(0, "/opt/trn_rl_repo")

import numpy as np
import ml_dtypes

BF16 = ml_dtypes.bfloat16

# problem dims (hardcoded per harness contract)
B = 8192
OBS = 249
NB = 12
NA = 11
NODE = 45
GRP = 17
D0 = 62
D0P = 128            # L1 contraction padded to full PE height (avoids row_grp
                     # switches between 62-row and 128-row stationaries, which
                     # expose LDWEIGHTS on every L1<->L2/L3 transition)
D1 = 2048
D2 = 1024
D3 = 512

NCORES = 8
NGRP = 4             # branch groups
NBL = NB // NGRP     # 3 branches per core
LBH = B // 2         # 4096 rows per core (batch halves)
BT = 512             # batch tile
NBT = LBH // BT      # 8 batch tiles
M1 = D1 // 128       # 16 output tiles of layer 1
K2 = D1 // 128       # 16 contraction tiles of layer 2
M2 = D2 // 128       # 8
K3 = D2 // 128       # 8
M3 = D3 // 128       # 4
KH = D3 // 128       # 4
NAP = 12             # head width padded even

_NC_CACHE = {}
LAST_RESULT = None


def _build_nc():
    if "nc" in _NC_CACHE:
        return _NC_CACHE["nc"]
    from concourse import bacc
    import concourse.mybir as mybir
    import concourse.tile as tile

    f32 = mybir.dt.float32
    bf16 = mybir.dt.bfloat16
    Relu = mybir.ActivationFunctionType.Relu
    ADD = mybir.AluOpType.add
    MAX = mybir.AluOpType.max

    nc = bacc.Bacc("TRN2")

    px_d = nc.declare_dram_parameter("pxp", [NBL, D0P, LBH], bf16, isOutput=False)
    W1_d = nc.declare_dram_parameter("W1p", [NBL, D0P, D1], bf16, isOutput=False)
    W2_d = nc.declare_dram_parameter("W2p", [NBL, 128, K2, D2], bf16, isOutput=False)
    W3_d = nc.declare_dram_parameter("W3p", [NBL, 128, K3, D3], bf16, isOutput=False)
    Wq_d = nc.declare_dram_parameter("Wqp", [NBL, 128, KH, NAP], bf16, isOutput=False)
    b_d = nc.declare_dram_parameter("bp", [NBL, 128, M1 + M2 + M3], f32, isOutput=False)
    bq_d = nc.declare_dram_parameter("bqp", [NBL, NAP, 1], f32, isOutput=False)
    out_d = nc.declare_dram_parameter("out", [NBL, NAP, LBH], f32, isOutput=True)

    with tile.TileContext(nc) as tc:
        with (
            tc.tile_pool(name="wp1", bufs=2) as wp1,
            tc.tile_pool(name="wp2", bufs=2) as wp2,
            tc.tile_pool(name="wp3", bufs=2) as wp3,
            tc.tile_pool(name="wpq", bufs=2) as wpq,
            tc.tile_pool(name="bbp", bufs=2) as bbp,
            tc.tile_pool(name="pxp_s", bufs=3) as pxp,
            tc.tile_pool(name="h1p", bufs=2) as h1p,
            tc.tile_pool(name="actp", bufs=1) as actp,
            tc.tile_pool(name="osp", bufs=3) as osp,
            tc.tile_pool(name="psA", bufs=4, space="PSUM") as psA,
            tc.tile_pool(name="psS", bufs=4, space="PSUM") as psS,
        ):
            h2 = actp.tile([128, K3, BT], bf16, tag="h2")
            h3 = actp.tile([128, KH, BT], bf16, tag="h3")

            iters = [(br, bt) for br in range(NBL) for bt in range(NBT)]
            loaded = {}
            pxs = {}
            h1s = {}

            def load_branch(br):
                # branch 0 loads gate the pipeline start: spread them over
                # idle queues so W1/px land first and W2 chunks in parallel
                first = br == 0
                w_eng = [nc.gpsimd, nc.sync, nc.gpsimd, nc.sync] if first \
                    else [nc.sync] * 4
                w1t = wp1.tile([D0P, D1], bf16, tag="w1", name=f"w1_{br}")
                nc.sync.dma_start(w1t[:], W1_d[br])
                w2t = wp2.tile([128, K2, D2], bf16, tag="w2", name=f"w2_{br}")
                for c in range(4):
                    w_eng[c].dma_start(
                        w2t[:, 4 * c:4 * c + 4, :], W2_d[br][:, 4 * c:4 * c + 4, :]
                    )
                w3t = wp3.tile([128, K3, D3], bf16, tag="w3", name=f"w3_{br}")
                (nc.gpsimd if first else nc.sync).dma_start(w3t[:], W3_d[br])
                s_eng = nc.scalar if first else nc.sync
                wqt = wpq.tile([128, KH, NAP], bf16, tag="wq", name=f"wq_{br}")
                s_eng.dma_start(wqt[:], Wq_d[br])
                btile = bbp.tile([128, M1 + M2 + M3], f32, tag="b", name=f"b_{br}")
                s_eng.dma_start(btile[:], b_d[br])
                bqt = bbp.tile([NAP, 1], f32, tag="bq", name=f"bq_{br}")
                s_eng.dma_start(bqt[:], bq_d[br])
                loaded[br] = (w1t, w2t, w3t, wqt, btile, bqt)

            def load_px(idx):
                br, bt = iters[idx]
                bsl = slice(bt * BT, (bt + 1) * BT)
                px = pxp.tile([D0P, BT], bf16, tag="px", name=f"px_{idx}")
                nc.scalar.dma_start(px[:], px_d[br][:, bsl])
                pxs[idx] = px
                h1s[idx] = h1p.tile([128, K2, BT], bf16, tag="h1", name=f"h1_{idx}")

            def drain_split(dst, ps, bias, j):
                half = BT // 2
                a, b = dst[:, 0:half], dst[:, half:BT]
                pa, pb = ps[:, 0:half], ps[:, half:BT]
                if j % 2 == 0:
                    nc.scalar.activation(a, pa, Relu, bias=bias, scale=1.0)
                    nc.vector.tensor_scalar(b, pb, bias, 0.0, ADD, MAX)
                else:
                    nc.vector.tensor_scalar(a, pa, bias, 0.0, ADD, MAX)
                    nc.scalar.activation(b, pb, Relu, bias=bias, scale=1.0)

            def emit_L1_mm(idx, m):
                br, _ = iters[idx]
                w1t, _, _, _, btile, _ = loaded[br]
                ps = psS.tile([128, BT], f32, tag="ps", name=f"l1ps_{idx}_{m}")
                nc.tensor.matmul(
                    ps[:], w1t[:, m * 128:(m + 1) * 128], pxs[idx][:],
                    start=True, stop=True,
                )
                drain_split(h1s[idx][:, m, :], ps[:], btile[:, m:m + 1], m)

            def emit_head(idx):
                br, bt = iters[idx]
                _, _, _, wqt, _, bqt = loaded[br]
                psh = psS.tile([NAP, BT], f32, tag="ps", name=f"psh_{idx}")
                for k in range(KH):
                    nc.tensor.matmul(
                        psh[:], wqt[:, k, :], h3[:, k, :],
                        start=(k == 0), stop=(k == KH - 1),
                    )
                ot = osp.tile([NAP, BT], f32, tag="os", name=f"ot_{idx}")
                nc.vector.tensor_scalar_add(ot[:], psh[:], bqt[:])
                bsl = slice(bt * BT, (bt + 1) * BT)
                nc.gpsimd.dma_start(out_d[br][:, bsl], ot[:])

            # prologue: px + W1 must land first (they gate the first L1
            # matmuls); the rest of branch 0 streams in behind them
            load_px(0)
            load_branch(0)
            for m in range(M1):
                emit_L1_mm(0, m)

            for idx, (br, bt) in enumerate(iters):
                w1t, w2t, w3t, wqt, btile, bqt = loaded[br]
                h1 = h1s[idx]
                nxt = idx + 1
                have_nxt = nxt < len(iters)
                if have_nxt:
                    load_px(nxt)
                if bt == 0 and br + 1 < NBL:
                    load_branch(br + 1)

                # prev head at iteration top: its 4 matmuls cover the
                # ps3(idx-1) -> mg0 PSUM-bank drain latency
                if idx > 0:
                    emit_head(idx - 1)

                # ---- L2 [2048 -> 1024]: two m-groups of 4, k-outer ----
                for mg in range(2):
                    ps2 = [psA.tile([128, BT], f32, tag="ps",
                                    name=f"ps2_{idx}_{mg}_{j}") for j in range(4)]
                    for k in range(K2):
                        for j in range(4):
                            m = mg * 4 + j
                            nc.tensor.matmul(
                                ps2[j][:], w2t[:, k, m * 128:(m + 1) * 128],
                                h1[:, k, :],
                                start=(k == 0), stop=(k == K2 - 1),
                            )
                    for j in range(4):
                        m = mg * 4 + j
                        drain_split(h2[:, m, :], ps2[j][:],
                                    btile[:, M1 + m:M1 + m + 1], j)
                    # L1 fillers hide the m-group drain latency
                    if have_nxt:
                        for f in range(3):
                            emit_L1_mm(nxt, 3 * mg + f)

                # ---- L3 [1024 -> 512] k-outer + next L1 interleave ----
                ps3 = [psA.tile([128, BT], f32, tag="ps",
                                name=f"ps3_{idx}_{m}") for m in range(M3)]
                for k in range(K3):
                    for m in range(M3):
                        nc.tensor.matmul(
                            ps3[m][:], w3t[:, k, m * 128:(m + 1) * 128],
                            h2[:, k, :],
                            start=(k == 0), stop=(k == K3 - 1),
                        )
                    if have_nxt and k < 4:
                        emit_L1_mm(nxt, 6 + 2 * k)
                        emit_L1_mm(nxt, 7 + 2 * k)
                if have_nxt:
                    emit_L1_mm(nxt, 14)
                    emit_L1_mm(nxt, 15)
                for m in range(M3):
                    drain_split(h3[:, m, :], ps3[m][:],
                                btile[:, M1 + M2 + m:M1 + M2 + m + 1], m)

            emit_head(len(iters) - 1)

    nc.compile()
    _NC_CACHE["nc"] = nc
    return nc


def _pack_weights(W1, b1, W2, b2, W3, b3, Wv, bv, Wa, ba):
    f = np.float32
    W1p = np.zeros((NB, D0P, D1), BF16)                           # [12, 128, 2048]
    W1p[:, :D0, :] = np.ascontiguousarray(W1).astype(BF16)
    W2p = np.ascontiguousarray(
        W2.reshape(NB, K2, 128, D2).transpose(0, 2, 1, 3)).astype(BF16)
    W3p = np.ascontiguousarray(
        W3.reshape(NB, K3, 128, D3).transpose(0, 2, 1, 3)).astype(BF16)
    # fold dueling head: q = h @ (Wv + Wa - mean(Wa)) + (bv + ba - mean(ba))
    Wq = Wv + Wa - Wa.mean(axis=2, keepdims=True)                 # [12, 512, 11]
    bq = bv + ba - ba.mean(axis=1, keepdims=True)                 # [12, 11]
    Wq = np.concatenate([Wq, np.zeros((NB, D3, NAP - NA), Wq.dtype)], axis=2)
    bq = np.concatenate([bq, np.zeros((NB, NAP - NA), bq.dtype)], axis=1)
    Wqp = np.ascontiguousarray(
        Wq.reshape(NB, KH, 128, NAP).transpose(0, 2, 1, 3)).astype(BF16)
    bp = np.concatenate(
        [
            b1.reshape(NB, M1, 128).transpose(0, 2, 1),
            b2.reshape(NB, M2, 128).transpose(0, 2, 1),
            b3.reshape(NB, M3, 128).transpose(0, 2, 1),
        ],
        axis=2,
    ).astype(f)                                                   # [12, 128, 28]
    bqp = np.ascontiguousarray(bq[:, :, None], f)                 # [12, 12, 1]
    return W1p, W2p, W3p, Wqp, bp, bqp


def kernel(x, W1, b1, W2, b2, W3, b3, Wv, bv, Wa, ba):
    global LAST_RESULT
    from concourse.bass_utils import run_bass_kernel_spmd

    x = np.asarray(x, np.float32)
    args = [np.asarray(a, np.float32) for a in (W1, b1, W2, b2, W3, b3, Wv, bv, Wa, ba)]
    W1p, W2p, W3p, Wqp, bp, bqp = _pack_weights(*args)

    node = x[:, :NODE]                                  # [B, 45]
    groups = x[:, NODE:].reshape(B, NB, GRP)            # [B, 12, 17]

    nc = _build_nc()
    in_maps = []
    for c in range(NCORES):
        g, h = c // 2, c % 2
        rows = slice(h * LBH, (h + 1) * LBH)
        brs = slice(g * NBL, (g + 1) * NBL)
        pxc = np.zeros((NBL, D0P, LBH), BF16)
        nT = np.ascontiguousarray(node[rows].T).astype(BF16)
        for j in range(NBL):
            pxc[j, :NODE] = nT
            pxc[j, NODE:D0] = groups[rows, g * NBL + j].T
        in_maps.append({
            "pxp": pxc,
            "W1p": W1p[brs], "W2p": W2p[brs], "W3p": W3p[brs], "Wqp": Wqp[brs],
            "bp": bp[brs], "bqp": bqp[brs],
        })

    res = run_bass_kernel_spmd(nc, in_maps, list(range(NCORES)))
    LAST_RESULT = res

    out = np.empty((NB, B, NA), np.float32)
    for c in range(NCORES):
        g, h = c // 2, c % 2
        rows = slice(h * LBH, (h + 1) * LBH)
        oc = res.results[c]["out"]                      # [3, 12, 4096]
        for j in range(NBL):
            out[g * NBL + j, rows, :] = oc[j, :NA, :].T
    return out


# revision 16
# speedup vs baseline: 1.1418x; 1.0046x over previous
"""Trainium2 Bass kernel for nn_BranchingQNetwork (12-branch dueling Q-MLP).

Strategy: hybrid sharding — 4 branch-groups x 2 batch-halves over 8 cores.
Each core computes 3 branches for 4096 rows. All tensors bf16 (weights and
activations; fp32 PSUM accumulation), so the full 3-branch weight set
(~16 MB) is SBUF-resident, prefetched one branch ahead; matmuls never wait
on HBM. The dueling head (v + a - mean(a)) is linear and folded into a
single [512, 12] weight on the host; head output is produced action-major
[12, batch] on-chip and transposed on the host.

PSUM discipline: 4 "accumulator" banks (L2 m-groups of 4, then L3) +
4 "streaming" banks (L1 single-shot matmuls, head). L1 matmuls of the next
iteration are emitted as fillers around the L2/L3 accumulation phases to
hide PSUM-drain latency.
"""
import sys

sys.path.insert(0, "/opt/trn_rl_repo")

import numpy as np
import ml_dtypes

BF16 = ml_dtypes.bfloat16

# problem dims (hardcoded per harness contract)
B = 8192
OBS = 249
NB = 12
NA = 11
NODE = 45
GRP = 17
D0 = 62
D0P = 128            # L1 contraction padded to full PE height (avoids row_grp
                     # switches between 62-row and 128-row stationaries, which
                     # expose LDWEIGHTS on every L1<->L2/L3 transition)
D1 = 2048
D2 = 1024
D3 = 512

NCORES = 8
NGRP = 4             # branch groups
NBL = NB // NGRP     # 3 branches per core
LBH = B // 2         # 4096 rows per core (batch halves)
BT = 512             # batch tile
NBT = LBH // BT      # 8 batch tiles
M1 = D1 // 128       # 16 output tiles of layer 1
K2 = D1 // 128       # 16 contraction tiles of layer 2
M2 = D2 // 128       # 8
K3 = D2 // 128       # 8
M3 = D3 // 128       # 4
KH = D3 // 128       # 4
NAP = 12             # head width padded even

_NC_CACHE = {}
LAST_RESULT = None


def _build_nc():
    if "nc" in _NC_CACHE:
        return _NC_CACHE["nc"]
    from concourse import bacc
    import concourse.mybir as mybir
    import concourse.tile as tile

    f32 = mybir.dt.float32
    bf16 = mybir.dt.bfloat16
    Relu = mybir.ActivationFunctionType.Relu
    ADD = mybir.AluOpType.add
    MAX = mybir.AluOpType.max

    nc = bacc.Bacc("TRN2")

    px_d = nc.declare_dram_parameter("pxp", [NBL, D0P, LBH], bf16, isOutput=False)
    W1_d = nc.declare_dram_parameter("W1p", [NBL, D0P, D1], bf16, isOutput=False)
    W2_d = nc.declare_dram_parameter("W2p", [NBL, 128, K2, D2], bf16, isOutput=False)
    W3_d = nc.declare_dram_parameter("W3p", [NBL, 128, K3, D3], bf16, isOutput=False)
    Wq_d = nc.declare_dram_parameter("Wqp", [NBL, 128, KH, NAP], bf16, isOutput=False)
    b_d = nc.declare_dram_parameter("bp", [NBL, 128, M1 + M2 + M3], f32, isOutput=False)
    bq_d = nc.declare_dram_parameter("bqp", [NBL, NAP, 1], f32, isOutput=False)
    out_d = nc.declare_dram_parameter("out", [NBL, NAP, LBH], f32, isOutput=True)

    with tile.TileContext(nc) as tc:
        with (
            tc.tile_pool(name="wp1", bufs=2) as wp1,
            tc.tile_pool(name="wp2", bufs=2) as wp2,
            tc.tile_pool(name="wp3", bufs=2) as wp3,
            tc.tile_pool(name="wpq", bufs=2) as wpq,
            tc.tile_pool(name="bbp", bufs=2) as bbp,
            tc.tile_pool(name="pxp_s", bufs=3) as pxp,
            tc.tile_pool(name="h1p", bufs=2) as h1p,
            tc.tile_pool(name="actp", bufs=1) as actp,
            tc.tile_pool(name="osp", bufs=3) as osp,
            tc.tile_pool(name="psA", bufs=4, space="PSUM") as psA,
            tc.tile_pool(name="psS", bufs=4, space="PSUM") as psS,
        ):
            h2 = actp.tile([128, K3, BT], bf16, tag="h2")
            h3 = actp.tile([128, KH, BT], bf16, tag="h3")

            iters = [(br, bt) for br in range(NBL) for bt in range(NBT)]
            loaded = {}
            pxs = {}
            h1s = {}

            def load_branch(br):
                # branch 0 loads gate the pipeline start: spread them over
                # idle queues so W1/px land first and W2 chunks in parallel
                first = br == 0
                w1t = wp1.tile([D0P, D1], bf16, tag="w1", name=f"w1_{br}")
                nc.sync.dma_start(w1t[:], W1_d[br])
                s_eng = nc.scalar if first else nc.sync
                wqt = wpq.tile([128, KH, NAP], bf16, tag="wq", name=f"wq_{br}")
                s_eng.dma_start(wqt[:], Wq_d[br])
                btile = bbp.tile([128, M1 + M2 + M3], f32, tag="b", name=f"b_{br}")
                s_eng.dma_start(btile[:], b_d[br])
                bqt = bbp.tile([NAP, 1], f32, tag="bq", name=f"bq_{br}")
                s_eng.dma_start(bqt[:], bq_d[br])
                w2t = wp2.tile([128, K2, D2], bf16, tag="w2", name=f"w2_{br}")
                if first:
                    # branch 0 gates the pipeline: split W2 into 8 chunks on
                    # the two HWDGE queues so chunk k lands just ahead of its
                    # k-group in mg0 (gpsimd DMA is the slow SWDGE path)
                    for c in range(8):
                        eng = nc.sync if c % 2 == 0 else nc.scalar
                        eng.dma_start(
                            w2t[:, 2 * c:2 * c + 2, :],
                            W2_d[br][:, 2 * c:2 * c + 2, :],
                        )
                else:
                    for c in range(4):
                        nc.sync.dma_start(
                            w2t[:, 4 * c:4 * c + 4, :],
                            W2_d[br][:, 4 * c:4 * c + 4, :],
                        )
                w3t = wp3.tile([128, K3, D3], bf16, tag="w3", name=f"w3_{br}")
                (nc.scalar if first else nc.sync).dma_start(w3t[:], W3_d[br])
                loaded[br] = (w1t, w2t, w3t, wqt, btile, bqt)

            def load_px(idx):
                br, bt = iters[idx]
                bsl = slice(bt * BT, (bt + 1) * BT)
                px = pxp.tile([D0P, BT], bf16, tag="px", name=f"px_{idx}")
                nc.scalar.dma_start(px[:], px_d[br][:, bsl])
                pxs[idx] = px
                h1s[idx] = h1p.tile([128, K2, BT], bf16, tag="h1", name=f"h1_{idx}")

            def drain_split(dst, ps, bias, j):
                half = BT // 2
                a, b = dst[:, 0:half], dst[:, half:BT]
                pa, pb = ps[:, 0:half], ps[:, half:BT]
                if j % 2 == 0:
                    nc.scalar.activation(a, pa, Relu, bias=bias, scale=1.0)
                    nc.vector.tensor_scalar(b, pb, bias, 0.0, ADD, MAX)
                else:
                    nc.vector.tensor_scalar(a, pa, bias, 0.0, ADD, MAX)
                    nc.scalar.activation(b, pb, Relu, bias=bias, scale=1.0)

            def emit_L1_mm(idx, m):
                br, _ = iters[idx]
                w1t, _, _, _, btile, _ = loaded[br]
                ps = psS.tile([128, BT], f32, tag="ps", name=f"l1ps_{idx}_{m}")
                nc.tensor.matmul(
                    ps[:], w1t[:, m * 128:(m + 1) * 128], pxs[idx][:],
                    start=True, stop=True,
                )
                drain_split(h1s[idx][:, m, :], ps[:], btile[:, m:m + 1], m)

            def emit_head(idx):
                br, bt = iters[idx]
                _, _, _, wqt, _, bqt = loaded[br]
                psh = psS.tile([NAP, BT], f32, tag="ps", name=f"psh_{idx}")
                for k in range(KH):
                    nc.tensor.matmul(
                        psh[:], wqt[:, k, :], h3[:, k, :],
                        start=(k == 0), stop=(k == KH - 1),
                    )
                ot = osp.tile([NAP, BT], f32, tag="os", name=f"ot_{idx}")
                nc.vector.tensor_scalar_add(ot[:], psh[:], bqt[:])
                bsl = slice(bt * BT, (bt + 1) * BT)
                nc.gpsimd.dma_start(out_d[br][:, bsl], ot[:])

            # prologue: px + W1 must land first (they gate the first L1
            # matmuls); the rest of branch 0 streams in behind them
            load_px(0)
            load_branch(0)
            for m in range(M1):
                emit_L1_mm(0, m)

            for idx, (br, bt) in enumerate(iters):
                w1t, w2t, w3t, wqt, btile, bqt = loaded[br]
                h1 = h1s[idx]
                nxt = idx + 1
                have_nxt = nxt < len(iters)
                if have_nxt:
                    load_px(nxt)
                if bt == 0 and br + 1 < NBL:
                    load_branch(br + 1)

                # prev head at iteration top: its 4 matmuls cover the
                # ps3(idx-1) -> mg0 PSUM-bank drain latency
                if idx > 0:
                    emit_head(idx - 1)

                # ---- L2 [2048 -> 1024]: two m-groups of 4, k-outer ----
                for mg in range(2):
                    ps2 = [psA.tile([128, BT], f32, tag="ps",
                                    name=f"ps2_{idx}_{mg}_{j}") for j in range(4)]
                    for k in range(K2):
                        for j in range(4):
                            m = mg * 4 + j
                            nc.tensor.matmul(
                                ps2[j][:], w2t[:, k, m * 128:(m + 1) * 128],
                                h1[:, k, :],
                                start=(k == 0), stop=(k == K2 - 1),
                            )
                    for j in range(4):
                        m = mg * 4 + j
                        drain_split(h2[:, m, :], ps2[j][:],
                                    btile[:, M1 + m:M1 + m + 1], j)
                    # L1 fillers hide the m-group drain latency
                    if have_nxt:
                        for f in range(3):
                            emit_L1_mm(nxt, 3 * mg + f)

                # ---- L3 [1024 -> 512] k-outer + next L1 interleave ----
                ps3 = [psA.tile([128, BT], f32, tag="ps",
                                name=f"ps3_{idx}_{m}") for m in range(M3)]
                for k in range(K3):
                    for m in range(M3):
                        nc.tensor.matmul(
                            ps3[m][:], w3t[:, k, m * 128:(m + 1) * 128],
                            h2[:, k, :],
                            start=(k == 0), stop=(k == K3 - 1),
                        )
                    if have_nxt and k < 4:
                        emit_L1_mm(nxt, 6 + 2 * k)
                        emit_L1_mm(nxt, 7 + 2 * k)
                if have_nxt:
                    emit_L1_mm(nxt, 14)
                    emit_L1_mm(nxt, 15)
                for m in range(M3):
                    drain_split(h3[:, m, :], ps3[m][:],
                                btile[:, M1 + M2 + m:M1 + M2 + m + 1], m)

            emit_head(len(iters) - 1)

    nc.compile()
    _NC_CACHE["nc"] = nc
    return nc


def _pack_weights(W1, b1, W2, b2, W3, b3, Wv, bv, Wa, ba):
    f = np.float32
    W1p = np.zeros((NB, D0P, D1), BF16)                           # [12, 128, 2048]
    W1p[:, :D0, :] = np.ascontiguousarray(W1).astype(BF16)
    W2p = np.ascontiguousarray(
        W2.reshape(NB, K2, 128, D2).transpose(0, 2, 1, 3)).astype(BF16)
    W3p = np.ascontiguousarray(
        W3.reshape(NB, K3, 128, D3).transpose(0, 2, 1, 3)).astype(BF16)
    # fold dueling head: q = h @ (Wv + Wa - mean(Wa)) + (bv + ba - mean(ba))
    Wq = Wv + Wa - Wa.mean(axis=2, keepdims=True)                 # [12, 512, 11]
    bq = bv + ba - ba.mean(axis=1, keepdims=True)                 # [12, 11]
    Wq = np.concatenate([Wq, np.zeros((NB, D3, NAP - NA), Wq.dtype)], axis=2)
    bq = np.concatenate([bq, np.zeros((NB, NAP - NA), bq.dtype)], axis=1)
    Wqp = np.ascontiguousarray(
        Wq.reshape(NB, KH, 128, NAP).transpose(0, 2, 1, 3)).astype(BF16)
    bp = np.concatenate(
        [
            b1.reshape(NB, M1, 128).transpose(0, 2, 1),
            b2.reshape(NB, M2, 128).transpose(0, 2, 1),
            b3.reshape(NB, M3, 128).transpose(0, 2, 1),
        ],
        axis=2,
    ).astype(f)                                                   # [12, 128, 28]
    bqp = np.ascontiguousarray(bq[:, :, None], f)                 # [12, 12, 1]
    return W1p, W2p, W3p, Wqp, bp, bqp


def kernel(x, W1, b1, W2, b2, W3, b3, Wv, bv, Wa, ba):
    global LAST_RESULT
    from concourse.bass_utils import run_bass_kernel_spmd

    x = np.asarray(x, np.float32)
    args = [np.asarray(a, np.float32) for a in (W1, b1, W2, b2, W3, b3, Wv, bv, Wa, ba)]
    W1p, W2p, W3p, Wqp, bp, bqp = _pack_weights(*args)

    node = x[:, :NODE]                                  # [B, 45]
    groups = x[:, NODE:].reshape(B, NB, GRP)            # [B, 12, 17]

    nc = _build_nc()
    in_maps = []
    for c in range(NCORES):
        g, h = c // 2, c % 2
        rows = slice(h * LBH, (h + 1) * LBH)
        brs = slice(g * NBL, (g + 1) * NBL)
        pxc = np.zeros((NBL, D0P, LBH), BF16)
        nT = np.ascontiguousarray(node[rows].T).astype(BF16)
        for j in range(NBL):
            pxc[j, :NODE] = nT
            pxc[j, NODE:D0] = groups[rows, g * NBL + j].T
        in_maps.append({
            "pxp": pxc,
            "W1p": W1p[brs], "W2p": W2p[brs], "W3p": W3p[brs], "Wqp": Wqp[brs],
            "bp": bp[brs], "bqp": bqp[brs],
        })

    res = run_bass_kernel_spmd(nc, in_maps, list(range(NCORES)))
    LAST_RESULT = res

    out = np.empty((NB, B, NA), np.float32)
    for c in range(NCORES):
        g, h = c // 2, c % 2
        rows = slice(h * LBH, (h + 1) * LBH)
        oc = res.results[c]["out"]                      # [3, 12, 4096]
        for j in range(NBL):
            out[g * NBL + j, rows, :] = oc[j, :NA, :].T
    return out


# revision 17
# speedup vs baseline: 1.1645x; 1.0199x over previous
"""Trainium2 Bass kernel for nn_BranchingQNetwork (12-branch dueling Q-MLP).

Strategy: hybrid sharding — 4 branch-groups x 2 batch-halves over 8 cores.
Each core computes 3 branches for 4096 rows. All tensors bf16 (fp32 PSUM
accumulation), so the full 3-branch weight set (~16 MB) is SBUF-resident,
prefetched one branch ahead; matmuls never wait on HBM. The dueling head
(v + a - mean(a)) is linear and folded into a single [512, 12] weight on
the host; head output is action-major [12, batch] on-chip, transposed on
the host.

PE schedule: batch tiles are processed in pairs of 512 columns sharing one
LDWEIGHTS (each stationary feeds two N=512 matmuls), which halves PE
weight-load traffic and doubles the completion-dependency margin. PSUM:
4 "accumulator" banks (L2/L3 m-groups of 2 x 2 halves, k-outer) + 4
"streaming" banks (L1, head). L1 matmuls of the next iteration and the
previous iteration's head are emitted as fillers at the accumulation-group
boundaries to hide PSUM-drain latency.
"""
import sys

sys.path.insert(0, "/opt/trn_rl_repo")

import numpy as np
import ml_dtypes

BF16 = ml_dtypes.bfloat16

# problem dims (hardcoded per harness contract)
B = 8192
OBS = 249
NB = 12
NA = 11
NODE = 45
GRP = 17
D0 = 62
D0P = 128            # L1 contraction zero-padded to full PE height (avoids
                     # row_grp switches that expose LDWEIGHTS on every
                     # L1<->L2/L3 transition)
D1 = 2048
D2 = 1024
D3 = 512

NCORES = 8
NGRP = 4             # branch groups
NBL = NB // NGRP     # 3 branches per core
LBH = B // 2         # 4096 rows per core (batch halves)
BT = 1024            # batch tile (pair of 512-column PSUM halves)
BH = 512             # PSUM half-tile
NBT = LBH // BT      # 4 batch tiles
M1 = D1 // 128       # 16 output tiles of layer 1
K2 = D1 // 128       # 16 contraction tiles of layer 2
M2 = D2 // 128       # 8
K3 = D2 // 128       # 8
M3 = D3 // 128       # 4
KH = D3 // 128       # 4
NAP = 12             # head width padded even

_NC_CACHE = {}
LAST_RESULT = None


def _build_nc():
    if "nc" in _NC_CACHE:
        return _NC_CACHE["nc"]
    from concourse import bacc
    import concourse.mybir as mybir
    import concourse.tile as tile

    f32 = mybir.dt.float32
    bf16 = mybir.dt.bfloat16
    Relu = mybir.ActivationFunctionType.Relu
    ADD = mybir.AluOpType.add
    MAX = mybir.AluOpType.max

    nc = bacc.Bacc("TRN2")

    px_d = nc.declare_dram_parameter("pxp", [NBL, D0P, LBH], bf16, isOutput=False)
    W1_d = nc.declare_dram_parameter("W1p", [NBL, D0P, D1], bf16, isOutput=False)
    W2_d = nc.declare_dram_parameter("W2p", [NBL, 128, K2, D2], bf16, isOutput=False)
    W3_d = nc.declare_dram_parameter("W3p", [NBL, 128, K3, D3], bf16, isOutput=False)
    Wq_d = nc.declare_dram_parameter("Wqp", [NBL, 128, KH, NAP], bf16, isOutput=False)
    b_d = nc.declare_dram_parameter("bp", [NBL, 128, M1 + M2 + M3], f32, isOutput=False)
    bq_d = nc.declare_dram_parameter("bqp", [NBL, NAP, 1], f32, isOutput=False)
    out_d = nc.declare_dram_parameter("out", [NBL, NAP, LBH], f32, isOutput=True)

    with tile.TileContext(nc) as tc:
        with (
            tc.tile_pool(name="wp1", bufs=2) as wp1,
            tc.tile_pool(name="wp2", bufs=2) as wp2,
            tc.tile_pool(name="wp3", bufs=2) as wp3,
            tc.tile_pool(name="wpq", bufs=2) as wpq,
            tc.tile_pool(name="bbp", bufs=2) as bbp,
            tc.tile_pool(name="pxp_s", bufs=2) as pxp,
            tc.tile_pool(name="h1p", bufs=2) as h1p,
            tc.tile_pool(name="actp", bufs=1) as actp,
            tc.tile_pool(name="osp", bufs=2) as osp,
            tc.tile_pool(name="psA", bufs=4, space="PSUM") as psA,
            tc.tile_pool(name="psS", bufs=4, space="PSUM") as psS,
        ):
            h2 = actp.tile([128, K3, BT], bf16, tag="h2")
            h3 = actp.tile([128, KH, BT], bf16, tag="h3")

            iters = [(br, bt) for br in range(NBL) for bt in range(NBT)]
            loaded = {}
            pxs = {}
            h1s = {}

            def load_branch(br):
                first = br == 0
                s_eng = nc.scalar if first else nc.sync
                w1t = wp1.tile([D0P, D1], bf16, tag="w1", name=f"w1_{br}")
                if first:
                    # branch 0 gates pipeline start: chunk W1/W2 across both
                    # HWDGE queues so slices land just ahead of their use
                    for c in range(4):
                        eng = nc.sync if c % 2 == 0 else nc.scalar
                        eng.dma_start(w1t[:, 512 * c:512 * (c + 1)],
                                      W1_d[br][:, 512 * c:512 * (c + 1)])
                else:
                    nc.sync.dma_start(w1t[:], W1_d[br])
                wqt = wpq.tile([128, KH, NAP], bf16, tag="wq", name=f"wq_{br}")
                s_eng.dma_start(wqt[:], Wq_d[br])
                btile = bbp.tile([128, M1 + M2 + M3], f32, tag="b", name=f"b_{br}")
                s_eng.dma_start(btile[:], b_d[br])
                bqt = bbp.tile([NAP, 1], f32, tag="bq", name=f"bq_{br}")
                s_eng.dma_start(bqt[:], bq_d[br])
                w2t = wp2.tile([128, K2, D2], bf16, tag="w2", name=f"w2_{br}")
                nch = 8 if first else 4
                kc = K2 // nch
                for c in range(nch):
                    eng = (nc.sync if c % 2 == 0 else nc.scalar) if first \
                        else nc.sync
                    eng.dma_start(
                        w2t[:, kc * c:kc * (c + 1), :],
                        W2_d[br][:, kc * c:kc * (c + 1), :],
                    )
                w3t = wp3.tile([128, K3, D3], bf16, tag="w3", name=f"w3_{br}")
                (nc.scalar if first else nc.sync).dma_start(w3t[:], W3_d[br])
                loaded[br] = (w1t, w2t, w3t, wqt, btile, bqt)

            def load_px(idx):
                br, bt = iters[idx]
                bsl = slice(bt * BT, (bt + 1) * BT)
                px = pxp.tile([D0P, BT], bf16, tag="px", name=f"px_{idx}")
                nc.scalar.dma_start(px[:], px_d[br][:, bsl])
                pxs[idx] = px
                h1s[idx] = h1p.tile([128, M1, BT], bf16, tag="h1", name=f"h1_{idx}")

            def drain_split(dst, ps, bias, j):
                a, b = dst[:, 0:BH // 2], dst[:, BH // 2:BH]
                pa, pb = ps[:, 0:BH // 2], ps[:, BH // 2:BH]
                if j % 2 == 0:
                    nc.scalar.activation(a, pa, Relu, bias=bias, scale=1.0)
                    nc.vector.tensor_scalar(b, pb, bias, 0.0, ADD, MAX)
                else:
                    nc.vector.tensor_scalar(a, pa, bias, 0.0, ADD, MAX)
                    nc.scalar.activation(b, pb, Relu, bias=bias, scale=1.0)

            def emit_L1_pair(idx, m):
                # one stationary, two N=512 matmuls (batch halves)
                br, _ = iters[idx]
                w1t, _, _, _, btile, _ = loaded[br]
                for h in range(2):
                    ps = psS.tile([128, BH], f32, tag="ps",
                                  name=f"l1ps_{idx}_{m}_{h}")
                    nc.tensor.matmul(
                        ps[:], w1t[:, m * 128:(m + 1) * 128],
                        pxs[idx][:, h * BH:(h + 1) * BH],
                        start=True, stop=True,
                    )
                    drain_split(h1s[idx][:, m, h * BH:(h + 1) * BH], ps[:],
                                btile[:, m:m + 1], m + h)

            def emit_head(idx):
                br, bt = iters[idx]
                _, _, _, wqt, _, bqt = loaded[br]
                psh = [psS.tile([NAP, BH], f32, tag="ps",
                                name=f"psh_{idx}_{h}") for h in range(2)]
                for k in range(KH):
                    for h in range(2):
                        nc.tensor.matmul(
                            psh[h][:], wqt[:, k, :],
                            h3[:, k, h * BH:(h + 1) * BH],
                            start=(k == 0), stop=(k == KH - 1),
                        )
                ot = osp.tile([NAP, BT], f32, tag="os", name=f"ot_{idx}")
                for h in range(2):
                    nc.vector.tensor_scalar_add(
                        ot[:, h * BH:(h + 1) * BH], psh[h][:], bqt[:])
                bsl = slice(bt * BT, (bt + 1) * BT)
                nc.gpsimd.dma_start(out_d[br][:, bsl], ot[:])

            # prologue: px + W1 land first; L1 of iteration 0 standalone
            load_px(0)
            load_branch(0)
            for m in range(M1):
                emit_L1_pair(0, m)

            for idx, (br, bt) in enumerate(iters):
                w1t, w2t, w3t, wqt, btile, bqt = loaded[br]
                h1 = h1s[idx]
                nxt = idx + 1
                have_nxt = nxt < len(iters)
                if have_nxt:
                    load_px(nxt)
                if bt == 0 and br + 1 < NBL:
                    load_branch(br + 1)

                fill = iter(range(M1))  # next-iteration L1 m indices

                def fillers(n):
                    if have_nxt:
                        for _ in range(n):
                            m = next(fill, None)
                            if m is not None:
                                emit_L1_pair(nxt, m)

                # prev head at iteration top: covers ps3(idx-1) -> L2 g0
                # PSUM drain latency
                if idx > 0:
                    emit_head(idx - 1)

                # ---- L2 [2048 -> 1024]: 4 m-groups of 2, k-outer,
                #      each stationary feeds both batch halves ----
                for g in range(4):
                    ps2 = [psA.tile([128, BH], f32, tag="ps",
                                    name=f"ps2_{idx}_{g}_{j}") for j in range(4)]
                    for k in range(K2):
                        for jm in range(2):
                            m = 2 * g + jm
                            for h in range(2):
                                nc.tensor.matmul(
                                    ps2[2 * jm + h][:],
                                    w2t[:, k, m * 128:(m + 1) * 128],
                                    h1[:, k, h * BH:(h + 1) * BH],
                                    start=(k == 0), stop=(k == K2 - 1),
                                )
                    for jm in range(2):
                        m = 2 * g + jm
                        for h in range(2):
                            drain_split(h2[:, m, h * BH:(h + 1) * BH],
                                        ps2[2 * jm + h][:],
                                        btile[:, M1 + m:M1 + m + 1], jm + h)
                    fillers(2)  # hide this group's drains

                # ---- L3 [1024 -> 512]: 2 m-groups of 2, k-outer ----
                for g in range(2):
                    ps3 = [psA.tile([128, BH], f32, tag="ps",
                                    name=f"ps3_{idx}_{g}_{j}") for j in range(4)]
                    for k in range(K3):
                        for jm in range(2):
                            m = 2 * g + jm
                            for h in range(2):
                                nc.tensor.matmul(
                                    ps3[2 * jm + h][:],
                                    w3t[:, k, m * 128:(m + 1) * 128],
                                    h2[:, k, h * BH:(h + 1) * BH],
                                    start=(k == 0), stop=(k == K3 - 1),
                                )
                        if k == 3:
                            fillers(2)  # spread remaining L1 work
                    for jm in range(2):
                        m = 2 * g + jm
                        for h in range(2):
                            drain_split(h3[:, m, h * BH:(h + 1) * BH],
                                        ps3[2 * jm + h][:],
                                        btile[:, M1 + M2 + m:M1 + M2 + m + 1],
                                        jm + h)
                    fillers(2)
                fillers(M1)  # flush any remaining next-L1 pairs

            emit_head(len(iters) - 1)

    nc.compile()
    _NC_CACHE["nc"] = nc
    return nc


def _pack_weights(W1, b1, W2, b2, W3, b3, Wv, bv, Wa, ba):
    f = np.float32
    W1p = np.zeros((NB, D0P, D1), BF16)                           # [12, 128, 2048]
    W1p[:, :D0, :] = np.ascontiguousarray(W1).astype(BF16)
    W2p = np.ascontiguousarray(
        W2.reshape(NB, K2, 128, D2).transpose(0, 2, 1, 3)).astype(BF16)
    W3p = np.ascontiguousarray(
        W3.reshape(NB, K3, 128, D3).transpose(0, 2, 1, 3)).astype(BF16)
    # fold dueling head: q = h @ (Wv + Wa - mean(Wa)) + (bv + ba - mean(ba))
    Wq = Wv + Wa - Wa.mean(axis=2, keepdims=True)                 # [12, 512, 11]
    bq = bv + ba - ba.mean(axis=1, keepdims=True)                 # [12, 11]
    Wq = np.concatenate([Wq, np.zeros((NB, D3, NAP - NA), Wq.dtype)], axis=2)
    bq = np.concatenate([bq, np.zeros((NB, NAP - NA), bq.dtype)], axis=1)
    Wqp = np.ascontiguousarray(
        Wq.reshape(NB, KH, 128, NAP).transpose(0, 2, 1, 3)).astype(BF16)
    bp = np.concatenate(
        [
            b1.reshape(NB, M1, 128).transpose(0, 2, 1),
            b2.reshape(NB, M2, 128).transpose(0, 2, 1),
            b3.reshape(NB, M3, 128).transpose(0, 2, 1),
        ],
        axis=2,
    ).astype(f)                                                   # [12, 128, 28]
    bqp = np.ascontiguousarray(bq[:, :, None], f)                 # [12, 12, 1]
    return W1p, W2p, W3p, Wqp, bp, bqp


def kernel(x, W1, b1, W2, b2, W3, b3, Wv, bv, Wa, ba):
    global LAST_RESULT
    from concourse.bass_utils import run_bass_kernel_spmd

    x = np.asarray(x, np.float32)
    args = [np.asarray(a, np.float32) for a in (W1, b1, W2, b2, W3, b3, Wv, bv, Wa, ba)]
    W1p, W2p, W3p, Wqp, bp, bqp = _pack_weights(*args)

    node = x[:, :NODE]                                  # [B, 45]
    groups = x[:, NODE:].reshape(B, NB, GRP)            # [B, 12, 17]

    nc = _build_nc()
    in_maps = []
    for c in range(NCORES):
        g, h = c // 2, c % 2
        rows = slice(h * LBH, (h + 1) * LBH)
        brs = slice(g * NBL, (g + 1) * NBL)
        pxc = np.zeros((NBL, D0P, LBH), BF16)
        nT = np.ascontiguousarray(node[rows].T).astype(BF16)
        for j in range(NBL):
            pxc[j, :NODE] = nT
            pxc[j, NODE:D0] = groups[rows, g * NBL + j].T
        in_maps.append({
            "pxp": pxc,
            "W1p": W1p[brs], "W2p": W2p[brs], "W3p": W3p[brs], "Wqp": Wqp[brs],
            "bp": bp[brs], "bqp": bqp[brs],
        })

    res = run_bass_kernel_spmd(nc, in_maps, list(range(NCORES)))
    LAST_RESULT = res

    out = np.empty((NB, B, NA), np.float32)
    for c in range(NCORES):
        g, h = c // 2, c % 2
        rows = slice(h * LBH, (h + 1) * LBH)
        oc = res.results[c]["out"]                      # [3, 12, 4096]
        for j in range(NBL):
            out[g * NBL + j, rows, :] = oc[j, :NA, :].T
    return out
